# revision 1
# baseline (speedup 1.0000x reference)
"""Hyperbolic MLR logits (Ganea et al.) on 8 Trainium2 NeuronCores.

Shapes (hardcoded): inp [128, 512] f32, p [2048, 512] f32, a [2048, 512] f32,
output [128, 2048] f32.

Math
----
With c = 1, u = -p, the reference is
    logit[b,k] = lam_p[k] * ||a_k|| * asinh( 2 <w, a_k> / (||a_k|| (1 - ||w||^2)) )
with w = u (+)_mobius x.  Using the gyrovector identity
    1 - ||u (+) x||^2 = (1 - ||u||^2)(1 - ||x||^2) / den,   den = 1 + 2<u,x> + ||u||^2 ||x||^2
the den factors cancel and the whole thing collapses to
    logit[b,k] = lam[k] * asinh( vws[b] * qscale[k] + <W_k, xs_b> )
where (all host-precomputable; winv folded into xs and vws):
    uu = ||p_k||^2, beta = 1 - uu, ua = -<p_k, a_k>, an = ||a_k||
    qscale = 2 ua / (an beta),  lam = 2 an / beta
    W_k = -2 qscale[k] p_k + (2 / an[k]) a_k          # folded weight, [K, D]
    vv = ||x_b||^2, winv = 1/(1 - vv)
    xs_b = x_b * winv[b],  vws = (1 + vv) * winv
So the device does one [B,D]x[D,K] matmul plus cheap elementwise work.

Precision: the matmul runs as a compensated bf16 split
    xs @ W.T ~= xh@Wh.T + xh@Wl.T + xl@Wh.T      (x = xh + xl, W = Wh + Wl)
which is fp32-grade (~4e-6) at bf16 matmul throughput (1 cycle/row on the PE
vs 4 for fp32) and the same DMA bytes as shipping fp32.

Sharding: K=2048 row-sharded over 8 cores (256 classes each), x replicated.
Per-core device program:
    DMA in:  aux [1, 768] f32      ([vws, ones, qscale_shard, lam_shard]; first)
             wx1 [128, 1536] bf16  (xh, xl chunks + Wh, Wl chunk 0)
             wc1..wc3 [128, 512] bf16  (per-d-chunk Wh_c | Wl_c)
    PE:      mm  = vws x qscale (fp32 rank-1, hidden behind DMA) + bf16 terms,
             split into two K-half PSUM tiles so the first half's elementwise
             chain overlaps the second half's matmuls
             lam_bc = ones x lam (fp32 rank-1 broadcast, hidden behind DMA)
    ACT/DVE: out = lam_bc * asinh(mm), asinh via ln/exp (one ACT table set)
    DMA out: out [128, 256] f32
"""

import os
import sys

import numpy as np

B, K, D = 128, 2048, 512
NCORES = 8
KLOC = K // NCORES          # 256 classes per core
DCH = D // 128              # 4 contraction chunks

_CACHE: dict = {}


def _import_concourse():
    try:
        import concourse.bass  # noqa: F401
    except ImportError:
        for path in ("/opt/trn_rl_repo", os.path.expanduser("~/.axon_site/_ro/trn_rl_repo")):
            if os.path.isdir(path) and path not in sys.path:
                sys.path.insert(0, path)
        import concourse.bass  # noqa: F401


def _build_nc(bench_loop=None):
    """Build the single-core Bass/Tile program (same program for all 8 cores).

    bench_loop=(n_iters, reps): wrap the body in a For_i hardware loop that
    executes it n_iters times with `reps` back-to-back copies per iteration
    (timing harness only -- RPC overhead and loop back-edge cost cancel in
    the delta between reps=1 and reps=2 builds at equal n_iters).
    """
    import concourse.tile as tile
    from concourse import bacc, mybir

    f32 = mybir.dt.float32
    bf16 = mybir.dt.bfloat16
    AF = mybir.ActivationFunctionType

    # wx1: xh chunks (512) + xl chunks (512) + Wh chunk 0 (KLOC) + Wl chunk 0
    # (KLOC).  wc1..wc3: per-d-chunk [Wh_c | Wl_c] so chunk-c matmuls start as
    # soon as that chunk's DMA lands.  Split so every instruction needs to
    # sync-wait on at most ONE semaphore (each trn2 instruction has a single
    # HW sync-wait slot, and every fresh data dependency -- even same-engine
    # RAW; the engines are pipelined with no SBUF interlock -- consumes it).
    WX1 = 2 * DCH * 128 + 2 * KLOC
    WCN = 2 * KLOC

    nc = bacc.Bacc("TRN2", target_bir_lowering=False, debug=False, num_devices=NCORES)
    aux_d = nc.declare_dram_parameter("aux", [1, 2 * 128 + 2 * KLOC], f32, isOutput=False)
    wx1_d = nc.declare_dram_parameter("wx1", [128, WX1], bf16, isOutput=False)
    wc_d = [
        nc.declare_dram_parameter(f"wc{c}", [128, WCN], bf16, isOutput=False)
        for c in range(1, DCH)
    ]
    out_d = nc.declare_dram_parameter("out", [128, KLOC], f32, isOutput=True)

    with tile.TileContext(nc) as tc:
        with (
            tc.tile_pool(name="sbuf", bufs=1) as pool,
            tc.tile_pool(name="psum", bufs=1, space="PSUM") as pp,
        ):
            # (aux DMA is emitted in emit() on the scalar ring BEFORE this
            # table load so its HWDGE descriptor-gen overlaps the sync ring's.)
            # Pre-place ONE activation-table load: set 6
            # (natural_log_exp_and_others) covers both Ln and Exp, so the
            # fixpoint table-load pass inserts nothing mid-kernel (vs 2x
            # ~1.3us set-thrash loads between ln/exp otherwise).
            nc.scalar.add_instruction(
                mybir.InstLoadActFuncSet(
                    name=nc.get_next_instruction_name(),
                    ins=[],
                    outs=[],
                    act_func_set_id=6,
                )
            )

            def emit():
                # aux is tiny (3KB) and gates the fp32 rank-1 matmul -- issue
                # it FIRST so it isn't queued behind the big transfers on the
                # shared SDMA engines.
                a_sb = pool.tile([1, 2 * 128 + 2 * KLOC], f32)
                nc.sync.dma_start(a_sb[:], aux_d[:])
                wx1_sb = pool.tile([128, WX1], bf16)
                nc.sync.dma_start(wx1_sb[:], wx1_d[:])
                wc_sb = []
                for c in range(1, DCH):
                    t = pool.tile([128, WCN], bf16, name=f"wc{c}", tag=f"wc{c}")
                    nc.sync.dma_start(t[:], wc_d[c - 1][:])
                    wc_sb.append(t)

                # aux layout: [0:128]=vws, [128:256]=ones, [256:256+KLOC]=qscale,
                # [256+KLOC:256+2*KLOC]=lam
                vw_ap = a_sb[:, 0:128]
                ones_ap = a_sb[:, 128:256]
                qs_ap = a_sb[:, 256 : 256 + KLOC]
                lam_ap = a_sb[:, 256 + KLOC : 256 + 2 * KLOC]

                # Everything downstream is split into two K-slices with
                # SEPARATE PSUM tiles (Tile tracks PSUM deps at bank
                # granularity, so separate banks let the first slice's
                # elementwise chain + store overlap the second slice's
                # matmuls).  The split is asymmetric: the last-released slice
                # is small so the unavoidable post-matmul chain tail runs on
                # short ops.
                # 2-way split at 80/176 -- swept 2..6-way and asymmetric
                # variants in the cost model; finer splits lose to per-matmul
                # overhead, coarser ones expose more chain latency.  Cuts must
                # be multiples of 16 (32-byte operand alignment for bf16).
                _cuts = [0, 80, KLOC]
                SPL = list(zip(_cuts[:-1], _cuts[1:]))

                # fp32 rank-1 vws x qscale halves: gated only by the tiny aux
                # DMA, they run while the big wx DMAs are in flight.  The lam
                # broadcast outer products likewise fill DMA-gated PE idle time;
                # emitting them before the mains also means the zcopy's PE wait
                # covers them for the final o muls.
                mm_h = [
                    pp.tile([128, b - a], f32, name=f"mm{h}", tag=f"mm{h}")
                    for h, (a, b) in enumerate(SPL)
                ]
                for h, (a, b) in enumerate(SPL):
                    nc.tensor.matmul(
                        mm_h[h][:], vw_ap, qs_ap[:, a:b], start=True, stop=False
                    )
                # lam broadcast is NOT on the L-half release path, so keep it as
                # one full-width outer product (hidden behind the wx DMAs).
                lam_ps = pp.tile([128, KLOC], f32)
                nc.tensor.matmul(lam_ps[:], ones_ap, lam_ap, start=True, stop=True)

                # Compensated bf16 main matmuls accumulating into the half PSUMs:
                # z[b,k] = vws[b] qscale[k] + sum_c xh_c Wh_c + xh_c Wl_c + xl_c Wh_c
                def xh(c):
                    return wx1_sb[:, c * 128 : (c + 1) * 128]

                def xl(c):
                    return wx1_sb[:, DCH * 128 + c * 128 : DCH * 128 + (c + 1) * 128]

                def wh(c):
                    if c == 0:
                        return wx1_sb[:, 2 * DCH * 128 : 2 * DCH * 128 + KLOC]
                    return wc_sb[c - 1][:, :KLOC]

                def wl(c):
                    if c == 0:
                        return wx1_sb[:, 2 * DCH * 128 + KLOC : 2 * DCH * 128 + 2 * KLOC]
                    return wc_sb[c - 1][:, KLOC:]

                for c in range(DCH):
                    terms = ((xh(c), wh(c)), (xh(c), wl(c)), (xl(c), wh(c)))
                    last = c == DCH - 1
                    # mid chunks: slices adjacent per term (shared lhsT stays
                    # loaded); last chunk: all first-slice terms first so it
                    # releases to the elementwise chain as early as possible.
                    NS = len(SPL)
                    order = (
                        [(t, h) for t in range(3) for h in range(NS)]
                        if not last
                        else [(t, h) for h in range(NS) for t in range(3)]
                    )
                    for t, h in order:
                        lhsT, rhs = terms[t]
                        a, b = SPL[h]
                        nc.tensor.matmul(
                            mm_h[h][:],
                            lhsT,
                            rhs[:, a:b],
                            start=False,
                            stop=(last and t == 2),
                            skip_group_check=True,
                        )

                # asinh chain per half, engine-scheduled so each op has one fresh
                # dep.  sqrt(1+z^2) is computed as exp(0.5*ln(1+z^2)) so that all
                # ACT transcendentals (Ln, Exp, Ln) come from ONE activation
                # table set (natural_log_exp_and_others) -- a single table load,
                # prefetched during the DMA phase, instead of a ~1.3us mid-chain
                # sqrt-table reload.
                #   DVE z  = copy(PSUM)    (waits PE, covers rank-1/lam too)
                #   DVE s  = z*z           (waits DVE self)
                #   ACT lw = ln(s + 1)     (waits DVE)
                #   ACT h  = exp(0.5*lw)   (ACT self)
                #   DVE t  = z + h         (waits ACT; z covered by DVE clock)
                #   ACT l  = ln(t)         (waits DVE)
                #   DVE o  = l * lam_bc    (waits ACT; lam PE dep covered)
                # The two Ln halves write one shared l tile; the final lam
                # multiply and the store then run full-width (one DMA setup +
                # one completion receipt on the tail instead of two).
                l_sb = pool.tile([128, KLOC], f32)
                for h, (a, b) in enumerate(SPL):
                    W = b - a
                    z_sb = pool.tile([128, W], f32, name=f"z{h}", tag=f"z{h}")
                    nc.vector.tensor_copy(z_sb[:], mm_h[h][:])
                    s_sb = pool.tile([128, W], f32, name=f"s{h}", tag=f"s{h}")
                    nc.vector.tensor_mul(s_sb[:], z_sb[:], z_sb[:])
                    lw_sb = pool.tile([128, W], f32, name=f"lw{h}", tag=f"lw{h}")
                    nc.scalar.activation(lw_sb[:], s_sb[:], AF.Ln, bias=1.0)
                    hh_sb = pool.tile([128, W], f32, name=f"hh{h}", tag=f"hh{h}")
                    nc.scalar.activation(hh_sb[:], lw_sb[:], AF.Exp, scale=0.5)
                    t_sb = pool.tile([128, W], f32, name=f"t{h}", tag=f"t{h}")
                    nc.vector.tensor_add(t_sb[:], z_sb[:], hh_sb[:])
                    nc.scalar.activation(l_sb[:, a:b], t_sb[:], AF.Ln)

                # Per-slice lam multiplies: o of the first slice runs while
                # the last slice's Ln is still on ACT; the single store waits
                # only the final o (earlier ones covered by the DVE clock).
                o_sb = pool.tile([128, KLOC], f32)
                for h, (a, b) in enumerate(SPL):
                    nc.vector.tensor_mul(
                        o_sb[:, a:b], l_sb[:, a:b], lam_ps[:, a:b]
                    )
                nc.sync.dma_start(out_d[:], o_sb[:])

            if bench_loop is None:
                emit()
            else:
                n_iters, reps = bench_loop
                with tc.For_i(0, n_iters, 1):
                    for _ in range(reps):
                        emit()

    nc.compile()
    return nc


def _host_prep(inp, p, a):
    """Host-side constant folding / layout prep. Returns per-core input maps."""
    import ml_dtypes

    bf = ml_dtypes.bfloat16
    inp64 = inp.astype(np.float64)
    p64 = p.astype(np.float64)
    a64 = a.astype(np.float64)

    vv = np.sum(inp64 * inp64, axis=1)            # [B]
    winv = 1.0 / (1.0 - vv)                       # [B]
    vws = (1.0 + vv) * winv                       # [B]

    uu = np.sum(p64 * p64, axis=1)                # [K]
    beta = 1.0 - uu
    ua = -np.sum(p64 * a64, axis=1)
    an = np.sqrt(np.sum(a64 * a64, axis=1))
    qscale = 2.0 * ua / (an * beta)               # [K]
    lam = 2.0 * an / beta                         # [K]
    W = (-2.0 * qscale)[:, None] * p64 + (2.0 / an)[:, None] * a64   # [K, D]

    xs = inp64 * winv[:, None]                    # [B, D]
    xs_h = xs.astype(bf)
    xs_l = (xs - xs_h.astype(np.float64)).astype(bf)

    def pack_x(m):  # [B, D] -> [128, DCH*128], chunk-major, d on partitions
        return np.ascontiguousarray(
            m.T.reshape(DCH, 128, B).transpose(1, 0, 2).reshape(128, DCH * B)
        )

    xh_p = pack_x(xs_h)
    xl_p = pack_x(xs_l)

    ones = np.ones(128, np.float64)
    in_maps = []
    for i in range(NCORES):
        k0 = i * KLOC
        Wc = W[k0 : k0 + KLOC]
        Wh = Wc.astype(bf)
        Wl = (Wc - Wh.astype(np.float64)).astype(bf)

        def pack_w(m):  # [KLOC, D] -> [128, DCH, KLOC], d on partitions
            return m.T.reshape(DCH, 128, KLOC).transpose(1, 0, 2)

        wh_p = pack_w(Wh)
        wl_p = pack_w(Wl)

        wx1 = np.empty((128, 2 * DCH * 128 + 2 * KLOC), bf)
        wx1[:, : DCH * 128] = xh_p
        wx1[:, DCH * 128 : 2 * DCH * 128] = xl_p
        wx1[:, 2 * DCH * 128 : 2 * DCH * 128 + KLOC] = wh_p[:, 0, :]
        wx1[:, 2 * DCH * 128 + KLOC :] = wl_p[:, 0, :]

        aux = np.concatenate(
            [vws, ones, qscale[k0 : k0 + KLOC], lam[k0 : k0 + KLOC]]
        ).astype(np.float32)[None, :]
        m = {"wx1": wx1, "aux": np.ascontiguousarray(aux)}
        for c in range(1, DCH):
            wc = np.empty((128, 2 * KLOC), bf)
            wc[:, :KLOC] = wh_p[:, c, :]
            wc[:, KLOC:] = wl_p[:, c, :]
            m[f"wc{c}"] = wc
        in_maps.append(m)
    return in_maps


def _run(in_maps, trace=False, **kw):
    from concourse.bass_utils import run_bass_kernel_spmd

    if "nc" not in _CACHE:
        _CACHE["nc"] = _build_nc()
    return run_bass_kernel_spmd(
        _CACHE["nc"], in_maps, list(range(NCORES)), trace=trace, **kw
    )


def kernel(inp, p, a):
    _import_concourse()
    inp = np.asarray(inp, np.float32)
    p = np.asarray(p, np.float32)
    a = np.asarray(a, np.float32)
    in_maps = _host_prep(inp, p, a)
    res = _run(in_maps)
    out = np.concatenate(
        [np.asarray(res.results[i]["out"]) for i in range(NCORES)], axis=1
    )
    return out.astype(np.float32)

